# revision 14
# baseline (speedup 1.0000x reference)
"""Trainium2 Bass kernel for an ALBERT-style seq2seq block (self-attn + cross-attn).

Sharding: 8 cores = (batch b in 0..3) x (decoder-row half in 0..1).
Each core computes its 512 decoder rows of the final output for its batch;
k/v/ek/ev projections are duplicated across the 2 cores sharing a batch
(zero inter-core communication).

On-chip layout is feature-major: activations live as [hidden, token] so every
matmul contracts over the partition axis. Softmax denominators come from a
fused [v_h | ones] stationary block (bf16 PV matmul, rows 64:128 = denom).

The attention inner loop is ACT-bound (one 1us exp per step vs ~0.65us of
matmul), which would leave the PE idle ~40% of each step and let the HAM
clock gate re-throttle it to 1.2 GHz.  To keep the PE dense, projection work
is split into ~2-matmul units that are paced one per attention step:
encoder-k fills self-attention, encoder-v (ot 4-7) plus the first half of
the cross out-projection fill cross-attention, encoder-v (ot 0-3) fills the
LN1 window.  Units carry (min_pair, deadline_pair) so reads/writes of shared
buffers interleave correctly with the attention stream.

All HBM-resident weights/activations are bf16; PSUM accumulation stays fp32.
A burst of junk matmuls at t=0 engages the HAM before real work arrives.
1/sqrt(var) is computed as exp(-0.5*ln(var)) so the ACT engine stays on a
single activation-table set (no reloads around the softmax exps).
"""

import sys

sys.path.insert(0, "/opt/trn_rl_repo")

import numpy as np

import concourse.bacc as bacc
import concourse.mybir as mybir
from concourse.bass_utils import run_bass_kernel_spmd
from concourse.masks import make_identity
from concourse.tile import TileContext

F32 = mybir.dt.float32
F32R = mybir.dt.float32r
BF16 = mybir.dt.bfloat16
AF = mybir.ActivationFunctionType
ALU = mybir.AluOpType

P = 128          # partitions
H = 1024         # hidden
NT = H // P      # 8 tiles over hidden
NH = 16          # heads
D = 64           # head dim
T = 1024         # sequence length (encoder and decoder)
R = 512          # decoder rows per core
B = 4
EPS = 1e-12
NPAIR = NH // 2


def build_kernel():
    nc = bacc.Bacc("TRN2", num_devices=8)

    decT = nc.declare_dram_parameter("decT", [P, NT, T], BF16, isOutput=False)
    dqT_d = nc.declare_dram_parameter("decqT", [P, NT, R], BF16, isOutput=False)
    encT = nc.declare_dram_parameter("encT", [P, NT, T], BF16, isOutput=False)
    wqT = nc.declare_dram_parameter("wqT", [NT, P, NT, P], BF16, isOutput=False)
    wkT = nc.declare_dram_parameter("wkT", [NT, P, NT, P], BF16, isOutput=False)
    wvT = nc.declare_dram_parameter("wvT", [NT, P, NT, P], BF16, isOutput=False)
    wdT = nc.declare_dram_parameter("wdT", [NT, P, NT, P], BF16, isOutput=False)
    bq_d = nc.declare_dram_parameter("bq", [P, NT, 1], F32, isOutput=False)
    bk_d = nc.declare_dram_parameter("bk", [P, NT, 1], F32, isOutput=False)
    bv_d = nc.declare_dram_parameter("bv", [P, NT, 1], F32, isOutput=False)
    bd_d = nc.declare_dram_parameter("bd", [P, NT, 1], F32, isOutput=False)
    lng_d = nc.declare_dram_parameter("lng", [P, NT, 1], F32, isOutput=False)
    lnb_d = nc.declare_dram_parameter("lnb", [P, NT, 1], F32, isOutput=False)
    mt_d = nc.declare_dram_parameter("mt", [P, NT, 1], F32, isOutput=False)
    ms_d = nc.declare_dram_parameter("ms", [P, NT, 1], F32, isOutput=False)
    out_d = nc.declare_dram_parameter("out", [P, NT, R], F32, isOutput=True)

    def col_view(d):  # pre-tiled [P, NT, 1]
        return d.ap()

    def wchunk_view(d, ot):  # pre-tiled weights [NT, P, NT, P] -> [P, NT, P]
        return d.ap()[ot]

    with TileContext(nc) as tc:
        with tc.tile_pool(name="base", bufs=1) as base:
            actT = base.tile([P, NT, T], BF16, tag="actT")
            q1T = base.tile([P, NT, R], F32R, tag="q1T")
            kT = base.tile([P, NT, T], BF16, tag="kT")
            qb = base.tile([P, NT, R], BF16, tag="qb")
            vi = base.tile([P, NT, NH, P], BF16, tag="vi")
            ctxn = base.tile([P, NT, R], BF16, tag="ctxn")
            resT = base.tile([P, NT, R], F32R, tag="resT")
            slfT = base.tile([P, NT, R], F32R, tag="slfT")
            bqc = base.tile([P, NT, 1], F32, tag="bqc")
            bkc = base.tile([P, NT, 1], F32, tag="bkc")
            bvc = base.tile([P, NT, 1], F32, tag="bvc")
            bdc = base.tile([P, NT, 1], F32, tag="bdc")
            gc = base.tile([P, NT, 1], F32, tag="gc")
            bc = base.tile([P, NT, 1], F32, tag="bc")
            mtc = base.tile([P, NT, 1], F32, tag="mtc")
            msc = base.tile([P, NT, 1], F32, tag="msc")
            ones1 = base.tile([P, 1], F32R, tag="ones1")
            onesr = base.tile([1, P], F32R, tag="onesr")
            epsc = base.tile([P, 1], F32, tag="epsc")
            ident = base.tile([P, P], BF16, tag="ident")
            muBs = base.tile([P, R], F32, tag="muBs")

            dma = nc.sync.dma_start

            # Persistent weight pools: reserving SBUF up front lets their
            # DMAs issue mid-phase instead of waiting for pool turnover.
            wdp = tc.alloc_tile_pool(name="wdp", bufs=2)
            wdp4 = tc.alloc_tile_pool(name="wdp4", bufs=3)
            wvp = tc.alloc_tile_pool(name="wvp", bufs=2)

            # ---- PE warm-up: junk matmuls engage the HAM clock gate while
            # the first weight/activation DMAs are still in flight.
            wrm = tc.alloc_tile_pool(name="wrm", bufs=1)
            wj = wrm.tile([P, R], BF16, tag="wj")
            nc.vector.memset(wj[:, :], 0.001)
            wps = tc.alloc_tile_pool(name="wps", bufs=1, space="PSUM")
            wpp = wps.tile([P, R], F32, tag="wpp")
            for i in range(16):
                nc.tensor.matmul(wpp[:, :], wj[:, 0:P], wj[:, :],
                                 start=True, stop=True)
            # release immediately: later PSUM users overlap this bank with a
            # WAR dependency on the junk matmuls (done by ~7us)
            wps.release()

            dqp = tc.alloc_tile_pool(name="dqp", bufs=1)
            dqT = dqp.tile([P, NT, R], BF16, tag="dqT")
            dma(out=dqT[:, :, :], in_=dqT_d.ap())
            dma(out=bqc[:, :, :], in_=col_view(bq_d))
            dma(out=bkc[:, :, :], in_=col_view(bk_d))
            dma(out=bvc[:, :, :], in_=col_view(bv_d))
            dma(out=bdc[:, :, :], in_=col_view(bd_d))
            dma(out=gc[:, :, :], in_=col_view(lng_d))
            dma(out=bc[:, :, :], in_=col_view(lnb_d))
            dma(out=mtc[:, :, :], in_=col_view(mt_d))
            dma(out=msc[:, :, :], in_=col_view(ms_d))
            nc.vector.memset(ones1[:, :].bitcast(F32), 1.0)
            nc.vector.memset(onesr[:, :].bitcast(F32), 1.0)
            nc.vector.memset(epsc[:, :], EPS)
            for st in range(NT):
                nc.gpsimd.memset(vi[:, st, :, D:P], 1.0)
            make_identity(nc, ident[:, :])

            # ---------------- shared building blocks ----------------

            def kproj_group(src, wk, ot, pspool, bias, uid):
                """Full [P,2,R] k-projection group for o-tile ot (coarse)."""
                pk = pspool.tile([P, 2, R], F32, tag="pk2", name=f"pk{uid}_{ot}")
                for tch in range(2):
                    tsl = slice(tch * R, (tch + 1) * R)
                    for it in range(NT):
                        nc.tensor.matmul(
                            pk[:, tch, :], wk[:, it, :], src[:, it, tsl],
                            start=(it == 0), stop=(it == NT - 1))
                nc.vector.tensor_scalar_add(
                    kT[:, ot, :].rearrange("p (a b) -> p a b", a=2),
                    pk[:, :, :], bias[:, ot, :])

            def vproj_group_units(src, ot, tch, wtile_fn, pvpool, ptpool,
                                  vtpool, uid, pv_bufs=2):
                """V projection for (ot, tch) as a list of ~2-matmul units:
                4x [2 MMs] -> evict vt -> 2x [2 transposes] -> vi copy."""
                st8 = {}
                tsl = slice(tch * R, (tch + 1) * R)

                def mk_mm(i0):
                    def f():
                        if i0 == 0:
                            st8["pv"] = pvpool.tile(
                                [P, R], F32, tag="fb", bufs=pv_bufs,
                                name=f"pv{uid}_{ot}_{tch}")
                        pv = st8["pv"]
                        wv = wtile_fn()
                        for it in (i0, i0 + 1):
                            nc.tensor.matmul(
                                pv[:, :], wv[:, it, :], src[:, it, tsl],
                                start=(it == 0), stop=(it == NT - 1))
                        if i0 == NT - 2:
                            vt = vtpool.tile([P, R], BF16, tag="vt",
                                             name=f"vt{uid}_{ot}_{tch}")
                            st8["vt"] = vt
                            nc.vector.tensor_scalar_add(
                                vt[:, :], pv[:, :], bvc[:, ot, :])
                    return f

                def mk_tr(half):
                    def f():
                        if half == 0:
                            st8["pt"] = ptpool.tile(
                                [P, 4, P], BF16, tag="pt4",
                                name=f"pt{uid}_{ot}_{tch}")
                        pt = st8["pt"]
                        vt = st8["vt"]
                        for bj in (2 * half, 2 * half + 1):
                            nc.tensor.transpose(
                                pt[:, bj, :], vt[:, bj * P:(bj + 1) * P],
                                ident[:, :])
                        if half == 1:
                            nc.vector.tensor_copy(
                                vi[:, tch * 4:(tch + 1) * 4,
                                   2 * ot:2 * ot + 2, 0:D],
                                pt[:, :, :].rearrange(
                                    "p b (h c) -> p b h c", c=D))
                    return f

                return [mk_mm(0), mk_mm(2), mk_mm(4), mk_mm(6),
                        mk_tr(0), mk_tr(1)]

            # ---------------- attention with per-step unit pacing ----------

            def attention(qsrc, mcol, units, uid, offset=0):
                """units: list of (min_pair, deadline_pair|None, fn).
                One unit is paced per (pair, step); deadline units are force
                drained before their pair starts."""
                ui = 0
                total = NPAIR * NT

                def pump(j, gs):
                    nonlocal ui
                    target = max(0, gs - offset + 1) * len(units) // max(
                        1, total - offset)
                    while ui < len(units):
                        mp, dl, fn = units[ui]
                        if mp > j:
                            break
                        forced = dl is not None and dl <= j
                        if ui >= target and not forced:
                            break
                        fn()
                        ui += 1

                with tc.tile_pool(name="prp", bufs=4) as prp, \
                     tc.tile_pool(name="rcp", bufs=3) as rcp, \
                     tc.tile_pool(name="psc", bufs=2, space="PSUM") as psc, \
                     tc.tile_pool(name="pcx", bufs=1, space="PSUM") as pcx:
                    for j in range(NPAIR):
                        pump(j, j * NT)  # deadline drain before pair j
                        c0 = pcx.tile([P, R], F32, tag="c0", name=f"c0{uid}_{j}")
                        c1 = pcx.tile([P, R], F32, tag="c1", name=f"c1{uid}_{j}")
                        probs = [None] * NT
                        for st in range(NT + 1):
                            # scores + exp for step st; PV for step st-1 (SW
                            # pipeline so the in-order PE stream never waits
                            # on the current exp)
                            if st < NT:
                                ssl = slice(st * P, (st + 1) * P)
                                s01 = psc.tile([P, 2, R], F32, tag="s01",
                                               name=f"s{uid}_{j}_{st}")
                                nc.tensor.matmul(
                                    s01[:, 0, :], kT[0:D, j, ssl], qsrc[0:D, j, :])
                                nc.tensor.matmul(
                                    s01[:, 1, :], kT[D:P, j, ssl], qsrc[D:P, j, :])
                                p01 = prp.tile([P, 2, R], BF16, tag="p01",
                                               name=f"p{uid}_{j}_{st}")
                                nc.scalar.activation(
                                    p01[:, :, :], s01[:, :, :], AF.Exp,
                                    bias=mcol[:, st, :], scale=0.125)
                                probs[st] = p01
                            if st > 0:
                                pp01 = probs[st - 1]
                                nc.tensor.matmul(
                                    c0[:, :], vi[:, st - 1, 2 * j, :], pp01[:, 0, :],
                                    start=(st == 1), stop=(st == NT))
                                nc.tensor.matmul(
                                    c1[:, :], vi[:, st - 1, 2 * j + 1, :], pp01[:, 1, :],
                                    start=(st == 1), stop=(st == NT))
                            if st > 0:
                                pump(j, j * NT + st)
                        # denominator extraction straight from PSUM (base-64
                        # slice), reciprocal, then normalize -> ctxn
                        d0 = rcp.tile([D, R], F32, tag="rr", bufs=4, name=f"d0{uid}_{j}")
                        d1 = rcp.tile([D, R], F32, tag="rr", bufs=4, name=f"d1{uid}_{j}")
                        r0 = rcp.tile([D, R], F32, tag="rr", bufs=4, name=f"r0{uid}_{j}")
                        r1 = rcp.tile([D, R], F32, tag="rr", bufs=4, name=f"r1{uid}_{j}")
                        nc.vector.tensor_copy(d0[:, :], c0[D:P, :])
                        nc.vector.reciprocal_approx_fast(r0[:, :], d0[:, :])
                        nc.vector.tensor_mul(ctxn[0:D, j, :], c0[0:D, :], r0[:, :])
                        nc.vector.tensor_copy(d1[:, :], c1[D:P, :])
                        nc.vector.reciprocal_approx_fast(r1[:, :], d1[:, :])
                        nc.vector.tensor_mul(ctxn[D:P, j, :], c1[0:D, :], r1[:, :])
                    while ui < len(units):
                        units[ui][2]()
                        ui += 1

            # ---------------- out-projection + layernorm ----------------

            def proj_ln(resid_src, dst, fillers, uid, partial=None, qcopy=None,
                        hold=0, store=None, tail_split=False):
                """Out-projection + residual into resT with LN stats fused
                per o-tile; then row stats, broadcast, per-o-tile apply -> dst.
                If partial is given, it holds ht 0..3 of the accumulation and
                only ht 4..7 run here.  The last `hold` fillers are emitted
                after the LN stat matmuls so the PE has work while the stat
                chain drains.  store: DMA each applied o-tile out.
                tail_split: run part of the apply on GpSimd (final tail)."""
                fill_i = 0
                with tc.tile_pool(name="sqp", bufs=2) as sqp, \
                     tc.tile_pool(name="lnp", bufs=1) as lnp, \
                     tc.tile_pool(name="ps3", bufs=2, space="PSUM") as ps, \
                     tc.tile_pool(name="ps4", bufs=1, space="PSUM") as ps4:
                    pmu = ps4.tile([1, R], F32, tag="pmu", name=f"pmu{uid}")
                    psq = ps4.tile([1, R], F32, tag="psq", name=f"psq{uid}")
                    for ot in range(NT):
                        if partial is None:
                            wd_c = wdp.tile([P, NT, P], BF16, tag="wd", name=f"wd{uid}_{ot}")
                            dma(out=wd_c[:, :, :], in_=wchunk_view(wdT, ot))
                            pp = ps.tile([P, R], F32, tag="pp", name=f"pp{uid}_{ot}")
                            for ht in range(NT):
                                nc.tensor.matmul(
                                    pp[:, :], wd_c[:, ht, :], ctxn[:, ht, :],
                                    start=(ht == 0), stop=(ht == NT - 1))
                            nc.vector.scalar_tensor_tensor(
                                resT[:, ot, :], pp[:, :], bdc[:, ot, :],
                                resid_src[:, ot, :].bitcast(F32), op0=ALU.add, op1=ALU.add)
                        else:
                            wd_c = wdp4.tile([P, 4, P], BF16, tag="wd4", name=f"wd{uid}_{ot}")
                            dma(out=wd_c[:, :, :], in_=wchunk_view(wdT, ot)[:, 4:NT, :])
                            pp = ps.tile([P, R], F32, tag="pp", name=f"pp{uid}_{ot}")
                            for ht in range(4):
                                nc.tensor.matmul(
                                    pp[:, :], wd_c[:, ht, :], ctxn[:, ht + 4, :],
                                    start=(ht == 0), stop=(ht == 3))
                            tsum = sqp.tile([P, R], F32, tag="tt", name=f"tsum{uid}_{ot}", bufs=2)
                            nc.vector.scalar_tensor_tensor(
                                tsum[:, :], pp[:, :], bdc[:, ot, :],
                                partial[:, ot, :], op0=ALU.add, op1=ALU.add)
                            nc.vector.tensor_add(
                                resT[:, ot, :], tsum[:, :],
                                resid_src[:, ot, :].bitcast(F32))
                        sq = sqp.tile([P, R], F32R, tag="sq", name=f"sq{uid}_{ot}")
                        nc.gpsimd.tensor_mul(
                            sq[:, :], resT[:, ot, :].bitcast(F32),
                            resT[:, ot, :].bitcast(F32))
                        nc.tensor.matmul(
                            pmu[:, :], ones1[:, :], resT[:, ot, :],
                            start=(ot == 0), stop=(ot == NT - 1))
                        nc.tensor.matmul(
                            psq[:, :], ones1[:, :], sq[:, :],
                            start=(ot == 0), stop=(ot == NT - 1))
                        early = len(fillers) - hold
                        while fillers and fill_i < (ot + 1) * early // NT:
                            fillers[fill_i]()
                            fill_i += 1
                    mu_r = lnp.tile([1, R], F32R, tag="lnrow", bufs=2, name=f"mu{uid}")
                    nc.scalar.mul(mu_r[:, :], pmu[:, :], 1.0 / H)
                    sq_r = lnp.tile([1, R], F32R, tag="lnrow", bufs=2, name=f"sqr{uid}")
                    nc.scalar.mul(sq_r[:, :], psq[:, :], 1.0 / H)
                    muB = ps4.tile([P, R], F32, tag="pmu", name=f"muBp{uid}")
                    nc.tensor.matmul(muB[:, :], onesr[:, :], mu_r[:, :])
                    sqBp = ps4.tile([P, R], F32, tag="psq", name=f"sqBp{uid}")
                    nc.tensor.matmul(sqBp[:, :], onesr[:, :], sq_r[:, :])
                    # held fillers: PE work covering the LN stat/apply chain
                    while fill_i < len(fillers):
                        fillers[fill_i]()
                        fill_i += 1
                    msB = sqp.tile([P, R], F32, tag="lnB", name=f"msB{uid}", bufs=2)
                    nc.scalar.square(msB[:, :], muB[:, :])
                    varB = sqp.tile([P, R], F32, tag="lnB", name=f"varB{uid}", bufs=2)
                    nc.vector.tensor_sub(varB[:, :], sqBp[:, :], msB[:, :])
                    # 1/sqrt(var+eps) as exp(-0.5*ln(var+eps)): Ln/Exp share
                    # one ACT table set, so no table reloads around softmax.
                    lnv = sqp.tile([P, R], F32, tag="lnB", name=f"lnv{uid}", bufs=2)
                    nc.scalar.activation(lnv[:, :], varB[:, :], AF.Ln, bias=epsc[:, :])
                    rsB = sqp.tile([P, R], F32, tag="rsB", name=f"rsB{uid}", bufs=1)
                    nc.scalar.activation(rsB[:, :], lnv[:, :], AF.Exp, scale=-0.5)
                    if tail_split:
                        nc.vector.tensor_copy(muBs[:, :], muB[:, :])
                    for ot in range(NT):
                        eng = nc.gpsimd if (tail_split and ot % 2 == 1) else nc.vector
                        mu_ap = muBs[:, :] if (tail_split and ot % 2 == 1) else muB[:, :]
                        t1 = sqp.tile([P, R], F32, tag="tt", name=f"t1{uid}_{ot}", bufs=2)
                        eng.tensor_sub(t1[:, :], resT[:, ot, :].bitcast(F32), mu_ap)
                        t2 = sqp.tile([P, R], F32, tag="tt", name=f"t2{uid}_{ot}", bufs=2)
                        eng.tensor_mul(t2[:, :], t1[:, :], rsB[:, :])
                        nc.scalar.activation(
                            dst[:, ot, :], t2[:, :], AF.Identity,
                            bias=bc[:, ot, :], scale=gc[:, ot, :])
                        if qcopy is not None:
                            nc.vector.tensor_copy(
                                qcopy[:, ot, :], dst[:, ot, :].bitcast(F32))
                        if store is not None:
                            dma(out=store.ap()[:, ot, :],
                                in_=dst[:, ot, :].bitcast(F32))

            # ================== phase 1: decoder projections ==================
            with tc.tile_pool(name="wp1", bufs=3) as wp1, \
                 tc.tile_pool(name="vt1", bufs=2) as vt1, \
                 tc.tile_pool(name="ps1", bufs=1, space="PSUM") as ps1, \
                 tc.tile_pool(name="pkv", bufs=2, space="PSUM") as pkv, \
                 tc.tile_pool(name="pst", bufs=2, space="PSUM") as pst:
                for ot in range(NT):
                    wq_c = wp1.tile([P, NT, P], BF16, tag="w", name=f"wq{ot}")
                    dma(out=wq_c[:, :, :], in_=wchunk_view(wqT, ot))
                    pq = ps1.tile([P, R], F32, tag="fb", bufs=2, name=f"pq{ot}")
                    for it in range(NT):
                        nc.tensor.matmul(
                            pq[:, :], wq_c[:, it, :], dqT[:, it, :],
                            start=(it == 0), stop=(it == NT - 1))
                    nc.vector.tensor_scalar_add(q1T[:, ot, :], pq[:, :], bqc[:, ot, :])
                    nc.vector.tensor_copy(qb[:, ot, :], q1T[:, ot, :].bitcast(F32))
                    if ot == 0:
                        dma(out=actT[:, :, :], in_=decT.ap())
                for ot in range(NT):
                    wk_c = wp1.tile([P, NT, P], BF16, tag="w", name=f"wk{ot}")
                    dma(out=wk_c[:, :, :], in_=wchunk_view(wkT, ot))
                    kproj_group(actT, wk_c, ot, pkv, bkc, "a")
                    wv_c = wp1.tile([P, NT, P], BF16, tag="w", name=f"wv{ot}")
                    dma(out=wv_c[:, :, :], in_=wchunk_view(wvT, ot))
                    for tch in range(2):
                        for u in vproj_group_units(actT, ot, tch,
                                                   (lambda w=wv_c: w),
                                                   ps1, pst, vt1, "a"):
                            u()

            dqp.release()
            wrm.release()

            # ============ phase 2: self-attn (+ encoder-k interleaved) ============
            # encT overwrites actT once all phase-1 reads are done.
            dma(out=actT[:, :, :], in_=encT.ap())
            with tc.tile_pool(name="wp2", bufs=3) as wp2, \
                 tc.tile_pool(name="ps2", bufs=1, space="PSUM") as ps2:
                ek_w = {}

                def ek_dma(ot):
                    wk = wp2.tile([P, NT, P], BF16, tag="wk2", name=f"wk2_{ot}")
                    dma(out=wk[:, :, :], in_=wchunk_view(wkT, ot))
                    ek_w[ot] = wk

                ek_dma(0)
                ek_dma(1)
                ek_pk = {}

                def mk_ek(ot, mi):
                    def f():
                        if mi == 0:
                            if ot + 2 < NT:
                                ek_dma(ot + 2)
                            ek_pk[ot] = ps2.tile([P, 2, R], F32, tag="pk2",
                                                 name=f"ekp{ot}")
                        pk = ek_pk[ot]
                        wk = ek_w[ot]
                        tch, i0 = divmod(2 * mi, NT)
                        tsl = slice(tch * R, (tch + 1) * R)
                        for it in (i0, i0 + 1):
                            nc.tensor.matmul(
                                pk[:, tch, :], wk[:, it, :], actT[:, it, tsl],
                                start=(it == 0), stop=(it == NT - 1))
                        if mi == 7:
                            nc.vector.tensor_scalar_add(
                                kT[:, ot, :].rearrange("p (a b) -> p a b", a=2),
                                pk[:, :, :], bkc[:, ot, :])
                    return f

                # 64 units, one per attention step; unit (ot, 7) rewrites
                # kT[ot] exactly after pair ot's last score read it.
                ek_units = [(0, None, mk_ek(ot, mi))
                            for ot in range(NT) for mi in range(8)]
                attention(qb, mtc, ek_units, "A", offset=2)

            # ========= phase 3: out-proj + LN1 (+ encoder-v ot 0-3) =========
            ev_w = {}

            def ev_dma(ot):
                wv = wvp.tile([P, NT, P], BF16, tag="wv2", name=f"wv2_{ot}")
                dma(out=wv[:, :, :], in_=wchunk_view(wvT, ot))
                ev_w[ot] = wv

            with tc.tile_pool(name="vt3", bufs=2) as vt3, \
                 tc.tile_pool(name="ps2b", bufs=1, space="PSUM") as ps2b, \
                 tc.tile_pool(name="pstb", bufs=1, space="PSUM") as pstb:
                ev_dma(0)
                ev_units = []
                for ot in range(4):
                    def pre(ot=ot):
                        if ot + 1 < 4:
                            ev_dma(ot + 1)
                    ev_units.append(pre)
                    for tch in range(2):
                        ev_units.extend(vproj_group_units(
                            actT, ot, tch, (lambda o=ot: ev_w[o]),
                            ps2b, pstb, vt3, f"b{ot}"))

                proj_ln(q1T, slfT, ev_units, "A", qcopy=qb, hold=14)

            # ==================== phase 4: cross-attention ====================
            # Fillers: encoder-v ot 4-7 (deadline: before pair ot reads vi),
            # then the first half (ht 0-3) of the cross out-projection.
            with tc.tile_pool(name="prt", bufs=1) as prt, \
                 tc.tile_pool(name="vt4", bufs=2) as vt4, \
                 tc.tile_pool(name="psB", bufs=1, space="PSUM") as psB, \
                 tc.tile_pool(name="pstc", bufs=1, space="PSUM") as pstc:
                partialA = prt.tile([P, NT, R], F32, tag="partialA")
                ev_dma(4)
                b_units = []
                for ot in range(4, NT):
                    def pre(ot=ot):
                        if ot + 1 < NT:
                            ev_dma(ot + 1)
                    b_units.append((0, ot, pre))
                    for tch in range(2):
                        b_units.extend(
                            (0, ot, u) for u in vproj_group_units(
                                actT, ot, tch, (lambda o=ot: ev_w[o]),
                                psB, pstc, vt4, f"c{ot}", pv_bufs=1))

                stage_w = {}
                stage_pp = {}

                def stage_dma(ot):
                    wd = wdp4.tile([P, 4, P], BF16, tag="wd4", name=f"wdA{ot}")
                    dma(out=wd[:, :, :], in_=wchunk_view(wdT, ot)[:, 0:4, :])
                    stage_w[ot] = wd

                def mk_stage(ot, half):
                    def f():
                        if half == 0:
                            if ot + 2 < NT:
                                stage_dma(ot + 2)
                            stage_pp[ot] = psB.tile([P, R], F32, tag="fb", bufs=1,
                                                    name=f"ppA{ot}")
                        pp = stage_pp[ot]
                        wd = stage_w[ot]
                        for ht in (2 * half, 2 * half + 1):
                            nc.tensor.matmul(
                                pp[:, :], wd[:, ht, :], ctxn[:, ht, :],
                                start=(ht == 0), stop=(ht == 3))
                        if half == 1:
                            nc.vector.tensor_copy(partialA[:, ot, :], pp[:, :])
                    return f

                # prefetch the first two wd4 chunks from mid-list (DMA only,
                # no PE work) so the first stage matmuls don't stall
                b_units.insert(27, (0, None, lambda: (stage_dma(0), stage_dma(1))))
                # cross out-proj ht 0-3 per o-tile; needs ctxn pairs 0-1
                # (ht 0,1) and 2-3, so gate on min_pair 4 for simplicity
                for ot in range(NT):
                    b_units.append((4, None, mk_stage(ot, 0)))
                    b_units.append((4, None, mk_stage(ot, 1)))

                attention(qb, msc, b_units, "B", offset=1)
                proj_ln(slfT, slfT, [], "B", partial=partialA, store=out_d,
                        tail_split=True)

            wvp.release()
            wdp4.release()
            wdp.release()

    nc.compile()
    return nc


_NC = None
import ml_dtypes

BF = ml_dtypes.bfloat16


def make_in_maps(encoder_states, decoder_inputs, src_attention_mask,
                 tgt_attention_mask, Wq, bq, Wk, bk, Wv, bv, Wd, bd, ln_g, ln_b):
    f = np.float32

    def wtile(w):  # [o,i] -> W.T tiled [ot, p, it, c], bf16
        return np.ascontiguousarray(
            np.asarray(w, f).T.reshape(NT, P, NT, P).transpose(2, 1, 0, 3)).astype(BF)

    def atile(x):  # [t,i] -> x.T tiled [p, it, t], bf16
        return np.ascontiguousarray(
            np.asarray(x, f).T.reshape(NT, P, -1).transpose(1, 0, 2)).astype(BF)

    wqT, wkT, wvT, wdT = wtile(Wq), wtile(Wk), wtile(Wv), wtile(Wd)
    col = lambda x: np.ascontiguousarray(
        np.asarray(x, f).reshape(NT, P).T.reshape(P, NT, 1))
    bq_, bk_, bv_, bd_ = col(bq), col(bk), col(bv), col(bd)
    g_, b_ = col(ln_g), col(ln_b)

    decT_b = [atile(decoder_inputs[b]) for b in range(B)]
    encT_b = [atile(encoder_states[b]) for b in range(B)]
    mt_b = [col(tgt_attention_mask[b, 0, 0, :]) for b in range(B)]
    ms_b = [col(src_attention_mask[b, 0, 0, :]) for b in range(B)]

    in_maps = []
    for c in range(8):
        b, half = c // 2, c % 2
        in_maps.append({
            "decT": decT_b[b],
            "decqT": np.ascontiguousarray(decT_b[b][:, :, half * R:(half + 1) * R]),
            "encT": encT_b[b],
            "wqT": wqT, "wkT": wkT, "wvT": wvT, "wdT": wdT,
            "bq": bq_, "bk": bk_, "bv": bv_, "bd": bd_,
            "lng": g_, "lnb": b_,
            "mt": mt_b[b], "ms": ms_b[b],
        })
    return in_maps


def kernel(**inputs):
    global _NC
    if _NC is None:
        _NC = build_kernel()
    nc = _NC
    in_maps = make_in_maps(**inputs)
    res = run_bass_kernel_spmd(nc, in_maps, core_ids=list(range(8)))
    out = np.empty((B, T, H), np.float32)
    for c in range(8):
        b, half = c // 2, c % 2
        buf = res.results[c]["out"]  # [p, ot, t]
        out[b, half * R:(half + 1) * R, :] = (
            buf.transpose(2, 1, 0).reshape(R, H))
    return out


# revision 15
# speedup vs baseline: 1.0020x; 1.0020x over previous
"""Trainium2 Bass kernel for an ALBERT-style seq2seq block (self-attn + cross-attn).

Sharding: 8 cores = (batch b in 0..3) x (decoder-row half in 0..1).
Each core computes its 512 decoder rows of the final output for its batch;
k/v/ek/ev projections are duplicated across the 2 cores sharing a batch
(zero inter-core communication).

On-chip layout is feature-major: activations live as [hidden, token] so every
matmul contracts over the partition axis. Softmax denominators come from a
fused [v_h | ones] stationary block (bf16 PV matmul, rows 64:128 = denom).

The attention inner loop is ACT-bound (one 1us exp per step vs ~0.65us of
matmul), which would leave the PE idle ~40% of each step and let the HAM
clock gate re-throttle it to 1.2 GHz.  To keep the PE dense, projection work
is split into ~2-matmul units that are paced one per attention step:
encoder-k fills self-attention, encoder-v (ot 4-7) plus the first half of
the cross out-projection fill cross-attention, encoder-v (ot 0-3) fills the
LN1 window.  Units carry (min_pair, deadline_pair) so reads/writes of shared
buffers interleave correctly with the attention stream.

All HBM-resident weights/activations are bf16; PSUM accumulation stays fp32.
A burst of junk matmuls at t=0 engages the HAM before real work arrives.
1/sqrt(var) is computed as exp(-0.5*ln(var)) so the ACT engine stays on a
single activation-table set (no reloads around the softmax exps).
"""

import sys

sys.path.insert(0, "/opt/trn_rl_repo")

import numpy as np

import concourse.bacc as bacc
import concourse.mybir as mybir
from concourse.bass_utils import run_bass_kernel_spmd
from concourse.masks import make_identity
from concourse.tile import TileContext

F32 = mybir.dt.float32
F32R = mybir.dt.float32r
BF16 = mybir.dt.bfloat16
AF = mybir.ActivationFunctionType
ALU = mybir.AluOpType

P = 128          # partitions
H = 1024         # hidden
NT = H // P      # 8 tiles over hidden
NH = 16          # heads
D = 64           # head dim
T = 1024         # sequence length (encoder and decoder)
R = 512          # decoder rows per core
B = 4
EPS = 1e-12
NPAIR = NH // 2


def build_kernel():
    nc = bacc.Bacc("TRN2", num_devices=8)

    decT = nc.declare_dram_parameter("decT", [P, NT, T], BF16, isOutput=False)
    dqT_d = nc.declare_dram_parameter("decqT", [P, NT, R], BF16, isOutput=False)
    encT = nc.declare_dram_parameter("encT", [P, NT, T], BF16, isOutput=False)
    wqT = nc.declare_dram_parameter("wqT", [NT, P, NT, P], BF16, isOutput=False)
    wkT = nc.declare_dram_parameter("wkT", [NT, P, NT, P], BF16, isOutput=False)
    wvT = nc.declare_dram_parameter("wvT", [NT, P, NT, P], BF16, isOutput=False)
    wdT = nc.declare_dram_parameter("wdT", [NT, P, NT, P], BF16, isOutput=False)
    bq_d = nc.declare_dram_parameter("bq", [P, NT, 1], F32, isOutput=False)
    bk_d = nc.declare_dram_parameter("bk", [P, NT, 1], F32, isOutput=False)
    bv_d = nc.declare_dram_parameter("bv", [P, NT, 1], F32, isOutput=False)
    bd_d = nc.declare_dram_parameter("bd", [P, NT, 1], F32, isOutput=False)
    lng_d = nc.declare_dram_parameter("lng", [P, NT, 1], F32, isOutput=False)
    lnb_d = nc.declare_dram_parameter("lnb", [P, NT, 1], F32, isOutput=False)
    mt_d = nc.declare_dram_parameter("mt", [P, NT, 1], F32, isOutput=False)
    ms_d = nc.declare_dram_parameter("ms", [P, NT, 1], F32, isOutput=False)
    out_d = nc.declare_dram_parameter("out", [P, NT, R], F32, isOutput=True)

    def col_view(d):  # pre-tiled [P, NT, 1]
        return d.ap()

    def wchunk_view(d, ot):  # pre-tiled weights [NT, P, NT, P] -> [P, NT, P]
        return d.ap()[ot]

    with TileContext(nc) as tc:
        with tc.tile_pool(name="base", bufs=1) as base:
            actT = base.tile([P, NT, T], BF16, tag="actT")
            q1T = base.tile([P, NT, R], F32R, tag="q1T")
            kT = base.tile([P, NT, T], BF16, tag="kT")
            qb = base.tile([P, NT, R], BF16, tag="qb")
            vi = base.tile([P, NT, NH, P], BF16, tag="vi")
            ctxn = base.tile([P, NT, R], BF16, tag="ctxn")
            resT = base.tile([P, NT, R], F32R, tag="resT")
            slfT = base.tile([P, NT, R], F32R, tag="slfT")
            bqc = base.tile([P, NT, 1], F32, tag="bqc")
            bkc = base.tile([P, NT, 1], F32, tag="bkc")
            bvc = base.tile([P, NT, 1], F32, tag="bvc")
            bdc = base.tile([P, NT, 1], F32, tag="bdc")
            gc = base.tile([P, NT, 1], F32, tag="gc")
            bc = base.tile([P, NT, 1], F32, tag="bc")
            mtc = base.tile([P, NT, 1], F32, tag="mtc")
            msc = base.tile([P, NT, 1], F32, tag="msc")
            ones1 = base.tile([P, 1], F32R, tag="ones1")
            onesr = base.tile([1, P], F32R, tag="onesr")
            epsc = base.tile([P, 1], F32, tag="epsc")
            ident = base.tile([P, P], BF16, tag="ident")
            muBs = base.tile([P, R], F32, tag="muBs")

            dma = nc.sync.dma_start

            # Persistent weight pools: reserving SBUF up front lets their
            # DMAs issue mid-phase instead of waiting for pool turnover.
            wdp = tc.alloc_tile_pool(name="wdp", bufs=2)
            wdp4 = tc.alloc_tile_pool(name="wdp4", bufs=3)
            wvp = tc.alloc_tile_pool(name="wvp", bufs=2)

            # ---- PE warm-up: junk matmuls engage the HAM clock gate while
            # the first weight/activation DMAs are still in flight.
            wrm = tc.alloc_tile_pool(name="wrm", bufs=1)
            wj = wrm.tile([P, R], BF16, tag="wj")
            nc.vector.memset(wj[:, :], 0.001)
            wps = tc.alloc_tile_pool(name="wps", bufs=1, space="PSUM")
            wpp = wps.tile([P, R], F32, tag="wpp")
            for i in range(16):
                nc.tensor.matmul(wpp[:, :], wj[:, 0:P], wj[:, :],
                                 start=True, stop=True)
            # release immediately: later PSUM users overlap this bank with a
            # WAR dependency on the junk matmuls (done by ~7us)
            wps.release()

            dqp = tc.alloc_tile_pool(name="dqp", bufs=1)
            dqT = dqp.tile([P, NT, R], BF16, tag="dqT")
            dma(out=dqT[:, :, :], in_=dqT_d.ap())
            dma(out=actT[:, :, :], in_=decT.ap())
            dma(out=bqc[:, :, :], in_=col_view(bq_d))
            dma(out=bkc[:, :, :], in_=col_view(bk_d))
            dma(out=bvc[:, :, :], in_=col_view(bv_d))
            dma(out=bdc[:, :, :], in_=col_view(bd_d))
            dma(out=gc[:, :, :], in_=col_view(lng_d))
            dma(out=bc[:, :, :], in_=col_view(lnb_d))
            dma(out=mtc[:, :, :], in_=col_view(mt_d))
            dma(out=msc[:, :, :], in_=col_view(ms_d))
            nc.vector.memset(ones1[:, :].bitcast(F32), 1.0)
            nc.vector.memset(onesr[:, :].bitcast(F32), 1.0)
            nc.vector.memset(epsc[:, :], EPS)
            for st in range(NT):
                nc.gpsimd.memset(vi[:, st, :, D:P], 1.0)
            make_identity(nc, ident[:, :])

            # ---------------- shared building blocks ----------------

            def kproj_group(src, wk, ot, pspool, bias, uid):
                """Full [P,2,R] k-projection group for o-tile ot (coarse)."""
                pk = pspool.tile([P, 2, R], F32, tag="pk2", name=f"pk{uid}_{ot}")
                for tch in range(2):
                    tsl = slice(tch * R, (tch + 1) * R)
                    for it in range(NT):
                        nc.tensor.matmul(
                            pk[:, tch, :], wk[:, it, :], src[:, it, tsl],
                            start=(it == 0), stop=(it == NT - 1))
                nc.vector.tensor_scalar_add(
                    kT[:, ot, :].rearrange("p (a b) -> p a b", a=2),
                    pk[:, :, :], bias[:, ot, :])

            def vproj_group_units(src, ot, tch, wtile_fn, pvpool, ptpool,
                                  vtpool, uid, pv_bufs=2):
                """V projection for (ot, tch) as a list of ~2-matmul units:
                4x [2 MMs] -> evict vt -> 2x [2 transposes] -> vi copy."""
                st8 = {}
                tsl = slice(tch * R, (tch + 1) * R)

                def mk_mm(i0):
                    def f():
                        if i0 == 0:
                            st8["pv"] = pvpool.tile(
                                [P, R], F32, tag="fb", bufs=pv_bufs,
                                name=f"pv{uid}_{ot}_{tch}")
                        pv = st8["pv"]
                        wv = wtile_fn()
                        for it in (i0, i0 + 1):
                            nc.tensor.matmul(
                                pv[:, :], wv[:, it, :], src[:, it, tsl],
                                start=(it == 0), stop=(it == NT - 1))
                        if i0 == NT - 2:
                            vt = vtpool.tile([P, R], BF16, tag="vt",
                                             name=f"vt{uid}_{ot}_{tch}")
                            st8["vt"] = vt
                            nc.vector.tensor_scalar_add(
                                vt[:, :], pv[:, :], bvc[:, ot, :])
                    return f

                def mk_tr(half):
                    def f():
                        if half == 0:
                            st8["pt"] = ptpool.tile(
                                [P, 4, P], BF16, tag="pt4",
                                name=f"pt{uid}_{ot}_{tch}")
                        pt = st8["pt"]
                        vt = st8["vt"]
                        for bj in (2 * half, 2 * half + 1):
                            nc.tensor.transpose(
                                pt[:, bj, :], vt[:, bj * P:(bj + 1) * P],
                                ident[:, :])
                        if half == 1:
                            nc.vector.tensor_copy(
                                vi[:, tch * 4:(tch + 1) * 4,
                                   2 * ot:2 * ot + 2, 0:D],
                                pt[:, :, :].rearrange(
                                    "p b (h c) -> p b h c", c=D))
                    return f

                return [mk_mm(0), mk_mm(2), mk_mm(4), mk_mm(6),
                        mk_tr(0), mk_tr(1)]

            # ---------------- attention with per-step unit pacing ----------

            def attention(qsrc, mcol, units, uid, offset=0):
                """units: list of (min_pair, deadline_pair|None, fn).
                One unit is paced per (pair, step); deadline units are force
                drained before their pair starts."""
                ui = 0
                total = NPAIR * NT

                def pump(j, gs):
                    nonlocal ui
                    target = max(0, gs - offset + 1) * len(units) // max(
                        1, total - offset)
                    while ui < len(units):
                        mp, dl, fn = units[ui]
                        if mp > j:
                            break
                        forced = dl is not None and dl <= j
                        if ui >= target and not forced:
                            break
                        fn()
                        ui += 1

                with tc.tile_pool(name="prp", bufs=4) as prp, \
                     tc.tile_pool(name="rcp", bufs=3) as rcp, \
                     tc.tile_pool(name="psc", bufs=2, space="PSUM") as psc, \
                     tc.tile_pool(name="pcx", bufs=1, space="PSUM") as pcx:
                    for j in range(NPAIR):
                        pump(j, j * NT)  # deadline drain before pair j
                        c0 = pcx.tile([P, R], F32, tag="c0", name=f"c0{uid}_{j}")
                        c1 = pcx.tile([P, R], F32, tag="c1", name=f"c1{uid}_{j}")
                        probs = [None] * NT
                        for st in range(NT + 1):
                            # scores + exp for step st; PV for step st-1 (SW
                            # pipeline so the in-order PE stream never waits
                            # on the current exp)
                            if st < NT:
                                ssl = slice(st * P, (st + 1) * P)
                                s01 = psc.tile([P, 2, R], F32, tag="s01",
                                               name=f"s{uid}_{j}_{st}")
                                nc.tensor.matmul(
                                    s01[:, 0, :], kT[0:D, j, ssl], qsrc[0:D, j, :])
                                nc.tensor.matmul(
                                    s01[:, 1, :], kT[D:P, j, ssl], qsrc[D:P, j, :])
                                p01 = prp.tile([P, 2, R], BF16, tag="p01",
                                               name=f"p{uid}_{j}_{st}")
                                nc.scalar.activation(
                                    p01[:, :, :], s01[:, :, :], AF.Exp,
                                    bias=mcol[:, st, :], scale=0.125)
                                probs[st] = p01
                            if st > 0:
                                pump(j, j * NT + st)
                                pp01 = probs[st - 1]
                                nc.tensor.matmul(
                                    c0[:, :], vi[:, st - 1, 2 * j, :], pp01[:, 0, :],
                                    start=(st == 1), stop=(st == NT))
                                nc.tensor.matmul(
                                    c1[:, :], vi[:, st - 1, 2 * j + 1, :], pp01[:, 1, :],
                                    start=(st == 1), stop=(st == NT))
                        # denominator extraction straight from PSUM (base-64
                        # slice), reciprocal, then normalize -> ctxn
                        d0 = rcp.tile([D, R], F32, tag="rr", bufs=4, name=f"d0{uid}_{j}")
                        d1 = rcp.tile([D, R], F32, tag="rr", bufs=4, name=f"d1{uid}_{j}")
                        r0 = rcp.tile([D, R], F32, tag="rr", bufs=4, name=f"r0{uid}_{j}")
                        r1 = rcp.tile([D, R], F32, tag="rr", bufs=4, name=f"r1{uid}_{j}")
                        nc.vector.tensor_copy(d0[:, :], c0[D:P, :])
                        nc.vector.reciprocal_approx_fast(r0[:, :], d0[:, :])
                        nc.vector.tensor_mul(ctxn[0:D, j, :], c0[0:D, :], r0[:, :])
                        nc.vector.tensor_copy(d1[:, :], c1[D:P, :])
                        nc.vector.reciprocal_approx_fast(r1[:, :], d1[:, :])
                        nc.vector.tensor_mul(ctxn[D:P, j, :], c1[0:D, :], r1[:, :])
                    while ui < len(units):
                        units[ui][2]()
                        ui += 1

            # ---------------- out-projection + layernorm ----------------

            def proj_ln(resid_src, dst, fillers, uid, partial=None, qcopy=None,
                        hold=0, store=None, tail_split=False, pre_wd=None):
                """Out-projection + residual into resT with LN stats fused
                per o-tile; then row stats, broadcast, per-o-tile apply -> dst.
                If partial is given, it holds ht 0..3 of the accumulation and
                only ht 4..7 run here.  The last `hold` fillers are emitted
                after the LN stat matmuls so the PE has work while the stat
                chain drains.  store: DMA each applied o-tile out.
                tail_split: run part of the apply on GpSimd (final tail)."""
                fill_i = 0
                with tc.tile_pool(name="sqp", bufs=2) as sqp, \
                     tc.tile_pool(name="lnp", bufs=1) as lnp, \
                     tc.tile_pool(name="ps3", bufs=2, space="PSUM") as ps, \
                     tc.tile_pool(name="ps4", bufs=1, space="PSUM") as ps4:
                    pmu = ps4.tile([1, R], F32, tag="pmu", name=f"pmu{uid}")
                    psq = ps4.tile([1, R], F32, tag="psq", name=f"psq{uid}")
                    for ot in range(NT):
                        if partial is None:
                            wd_c = wdp.tile([P, NT, P], BF16, tag="wd", name=f"wd{uid}_{ot}")
                            dma(out=wd_c[:, :, :], in_=wchunk_view(wdT, ot))
                            pp = ps.tile([P, R], F32, tag="pp", name=f"pp{uid}_{ot}")
                            for ht in range(NT):
                                nc.tensor.matmul(
                                    pp[:, :], wd_c[:, ht, :], ctxn[:, ht, :],
                                    start=(ht == 0), stop=(ht == NT - 1))
                            nc.vector.scalar_tensor_tensor(
                                resT[:, ot, :], pp[:, :], bdc[:, ot, :],
                                resid_src[:, ot, :].bitcast(F32), op0=ALU.add, op1=ALU.add)
                        else:
                            if pre_wd is not None and ot in pre_wd:
                                wd_c = pre_wd[ot]
                            else:
                                wd_c = wdp4.tile([P, 4, P], BF16, tag="wd4",
                                                 name=f"wd{uid}_{ot}")
                                dma(out=wd_c[:, :, :],
                                    in_=wchunk_view(wdT, ot)[:, 4:NT, :])
                            pp = ps.tile([P, R], F32, tag="pp", name=f"pp{uid}_{ot}")
                            for ht in range(4):
                                nc.tensor.matmul(
                                    pp[:, :], wd_c[:, ht, :], ctxn[:, ht + 4, :],
                                    start=(ht == 0), stop=(ht == 3))
                            tsum = sqp.tile([P, R], F32, tag="tt", name=f"tsum{uid}_{ot}", bufs=2)
                            nc.vector.scalar_tensor_tensor(
                                tsum[:, :], pp[:, :], bdc[:, ot, :],
                                partial[:, ot, :], op0=ALU.add, op1=ALU.add)
                            nc.vector.tensor_add(
                                resT[:, ot, :], tsum[:, :],
                                resid_src[:, ot, :].bitcast(F32))
                        sq = sqp.tile([P, R], F32R, tag="sq", name=f"sq{uid}_{ot}")
                        nc.gpsimd.tensor_mul(
                            sq[:, :], resT[:, ot, :].bitcast(F32),
                            resT[:, ot, :].bitcast(F32))
                        nc.tensor.matmul(
                            pmu[:, :], ones1[:, :], resT[:, ot, :],
                            start=(ot == 0), stop=(ot == NT - 1))
                        nc.tensor.matmul(
                            psq[:, :], ones1[:, :], sq[:, :],
                            start=(ot == 0), stop=(ot == NT - 1))
                        early = len(fillers) - hold
                        while fillers and fill_i < (ot + 1) * early // NT:
                            fillers[fill_i]()
                            fill_i += 1
                    mu_r = lnp.tile([1, R], F32R, tag="lnrow", bufs=2, name=f"mu{uid}")
                    nc.scalar.mul(mu_r[:, :], pmu[:, :], 1.0 / H)
                    sq_r = lnp.tile([1, R], F32R, tag="lnrow", bufs=2, name=f"sqr{uid}")
                    nc.scalar.mul(sq_r[:, :], psq[:, :], 1.0 / H)
                    muB = ps4.tile([P, R], F32, tag="pmu", name=f"muBp{uid}")
                    nc.tensor.matmul(muB[:, :], onesr[:, :], mu_r[:, :])
                    sqBp = ps4.tile([P, R], F32, tag="psq", name=f"sqBp{uid}")
                    nc.tensor.matmul(sqBp[:, :], onesr[:, :], sq_r[:, :])
                    # held fillers: PE work covering the LN stat/apply chain
                    while fill_i < len(fillers):
                        fillers[fill_i]()
                        fill_i += 1
                    msB = sqp.tile([P, R], F32, tag="lnB", name=f"msB{uid}", bufs=2)
                    nc.scalar.square(msB[:, :], muB[:, :])
                    varB = sqp.tile([P, R], F32, tag="lnB", name=f"varB{uid}", bufs=2)
                    nc.vector.tensor_sub(varB[:, :], sqBp[:, :], msB[:, :])
                    # 1/sqrt(var+eps) as exp(-0.5*ln(var+eps)): Ln/Exp share
                    # one ACT table set, so no table reloads around softmax.
                    lnv = sqp.tile([P, R], F32, tag="lnB", name=f"lnv{uid}", bufs=2)
                    nc.scalar.activation(lnv[:, :], varB[:, :], AF.Ln, bias=epsc[:, :])
                    rsB = sqp.tile([P, R], F32, tag="rsB", name=f"rsB{uid}", bufs=1)
                    nc.scalar.activation(rsB[:, :], lnv[:, :], AF.Exp, scale=-0.5)
                    if tail_split:
                        nc.vector.tensor_copy(muBs[:, :], muB[:, :])
                    for ot in range(NT):
                        on_gps = tail_split and ot % 3 == 1
                        eng = nc.gpsimd if on_gps else nc.vector
                        mu_ap = muBs[:, :] if on_gps else muB[:, :]
                        tg = "ttg" if on_gps else "ttd"
                        t1 = sqp.tile([P, R], F32, tag=tg, name=f"t1{uid}_{ot}", bufs=2)
                        eng.tensor_sub(t1[:, :], resT[:, ot, :].bitcast(F32), mu_ap)
                        t2 = sqp.tile([P, R], F32, tag=tg, name=f"t2{uid}_{ot}", bufs=2)
                        eng.tensor_mul(t2[:, :], t1[:, :], rsB[:, :])
                        nc.scalar.activation(
                            dst[:, ot, :], t2[:, :], AF.Identity,
                            bias=bc[:, ot, :], scale=gc[:, ot, :])
                        if qcopy is not None:
                            nc.vector.tensor_copy(
                                qcopy[:, ot, :], dst[:, ot, :].bitcast(F32))
                        if store is not None:
                            dma(out=store.ap()[:, ot, :],
                                in_=dst[:, ot, :].bitcast(F32))

            # ================== phase 1: decoder projections ==================
            with tc.tile_pool(name="wp1", bufs=3) as wp1, \
                 tc.tile_pool(name="vt1", bufs=2) as vt1, \
                 tc.tile_pool(name="ps1", bufs=1, space="PSUM") as ps1, \
                 tc.tile_pool(name="pkv", bufs=2, space="PSUM") as pkv, \
                 tc.tile_pool(name="pst", bufs=2, space="PSUM") as pst:
                for ot in range(NT):
                    wq_c = wp1.tile([P, NT, P], BF16, tag="w", name=f"wq{ot}")
                    dma(out=wq_c[:, :, :], in_=wchunk_view(wqT, ot))
                    pq = ps1.tile([P, R], F32, tag="fb", bufs=2, name=f"pq{ot}")
                    for it in range(NT):
                        nc.tensor.matmul(
                            pq[:, :], wq_c[:, it, :], dqT[:, it, :],
                            start=(it == 0), stop=(it == NT - 1))
                    nc.vector.tensor_scalar_add(q1T[:, ot, :], pq[:, :], bqc[:, ot, :])
                    nc.vector.tensor_copy(qb[:, ot, :], q1T[:, ot, :].bitcast(F32))
                for ot in range(NT):
                    wk_c = wp1.tile([P, NT, P], BF16, tag="w", name=f"wk{ot}")
                    dma(out=wk_c[:, :, :], in_=wchunk_view(wkT, ot))
                    kproj_group(actT, wk_c, ot, pkv, bkc, "a")
                    wv_c = wp1.tile([P, NT, P], BF16, tag="w", name=f"wv{ot}")
                    dma(out=wv_c[:, :, :], in_=wchunk_view(wvT, ot))
                    for tch in range(2):
                        for u in vproj_group_units(actT, ot, tch,
                                                   (lambda w=wv_c: w),
                                                   ps1, pst, vt1, "a"):
                            u()

            dqp.release()
            wrm.release()

            # ============ phase 2: self-attn (+ encoder-k interleaved) ============
            # encT overwrites actT once all phase-1 reads are done.
            dma(out=actT[:, :, :], in_=encT.ap())
            with tc.tile_pool(name="wp2", bufs=3) as wp2, \
                 tc.tile_pool(name="ps2", bufs=1, space="PSUM") as ps2:
                ek_w = {}

                def ek_dma(ot):
                    wk = wp2.tile([P, NT, P], BF16, tag="wk2", name=f"wk2_{ot}")
                    dma(out=wk[:, :, :], in_=wchunk_view(wkT, ot))
                    ek_w[ot] = wk

                ek_dma(0)
                ek_dma(1)
                ek_pk = {}

                def mk_ek(ot, mi):
                    def f():
                        if mi == 0:
                            if ot + 2 < NT:
                                ek_dma(ot + 2)
                            ek_pk[ot] = ps2.tile([P, 2, R], F32, tag="pk2",
                                                 name=f"ekp{ot}")
                        pk = ek_pk[ot]
                        wk = ek_w[ot]
                        tch, i0 = divmod(2 * mi, NT)
                        tsl = slice(tch * R, (tch + 1) * R)
                        for it in (i0, i0 + 1):
                            nc.tensor.matmul(
                                pk[:, tch, :], wk[:, it, :], actT[:, it, tsl],
                                start=(it == 0), stop=(it == NT - 1))
                        if mi == 7:
                            nc.vector.tensor_scalar_add(
                                kT[:, ot, :].rearrange("p (a b) -> p a b", a=2),
                                pk[:, :, :], bkc[:, ot, :])
                    return f

                # 64 units, one per attention step; unit (ot, 7) rewrites
                # kT[ot] exactly after pair ot's last score read it.
                ek_units = [(0, None, mk_ek(ot, mi))
                            for ot in range(NT) for mi in range(8)]
                attention(qb, mtc, ek_units, "A", offset=2)

            # ========= phase 3: out-proj + LN1 (+ encoder-v ot 0-3) =========
            ev_w = {}

            def ev_dma(ot):
                wv = wvp.tile([P, NT, P], BF16, tag="wv2", name=f"wv2_{ot}")
                dma(out=wv[:, :, :], in_=wchunk_view(wvT, ot))
                ev_w[ot] = wv

            with tc.tile_pool(name="vt3", bufs=2) as vt3, \
                 tc.tile_pool(name="ps2b", bufs=1, space="PSUM") as ps2b, \
                 tc.tile_pool(name="pstb", bufs=1, space="PSUM") as pstb:
                ev_dma(0)
                ev_units = []
                for ot in range(4):
                    def pre(ot=ot):
                        if ot + 1 < 4:
                            ev_dma(ot + 1)
                    ev_units.append(pre)
                    for tch in range(2):
                        ev_units.extend(vproj_group_units(
                            actT, ot, tch, (lambda o=ot: ev_w[o]),
                            ps2b, pstb, vt3, f"b{ot}"))

                proj_ln(q1T, slfT, ev_units, "A", qcopy=qb, hold=20)

            # ==================== phase 4: cross-attention ====================
            # Fillers: encoder-v ot 4-7 (deadline: before pair ot reads vi),
            # then the first half (ht 0-3) of the cross out-projection.
            with tc.tile_pool(name="prt", bufs=1) as prt, \
                 tc.tile_pool(name="vt4", bufs=2) as vt4, \
                 tc.tile_pool(name="psB", bufs=1, space="PSUM") as psB, \
                 tc.tile_pool(name="pstc", bufs=1, space="PSUM") as pstc:
                partialA = prt.tile([P, NT, R], F32, tag="partialA")
                ev_dma(4)
                b_units = []
                for ot in range(4, NT):
                    def pre(ot=ot):
                        if ot + 1 < NT:
                            ev_dma(ot + 1)
                    b_units.append((0, ot, pre))
                    for tch in range(2):
                        b_units.extend(
                            (0, ot, u) for u in vproj_group_units(
                                actT, ot, tch, (lambda o=ot: ev_w[o]),
                                psB, pstc, vt4, f"c{ot}", pv_bufs=1))

                stage_w = {}
                stage_pp = {}

                def stage_dma(ot):
                    wd = wdp4.tile([P, 4, P], BF16, tag="wd4", name=f"wdA{ot}")
                    dma(out=wd[:, :, :], in_=wchunk_view(wdT, ot)[:, 0:4, :])
                    stage_w[ot] = wd

                def mk_stage(ot, half):
                    def f():
                        if half == 0:
                            if ot + 2 < NT:
                                stage_dma(ot + 2)
                            stage_pp[ot] = psB.tile([P, R], F32, tag="fb", bufs=1,
                                                    name=f"ppA{ot}")
                        pp = stage_pp[ot]
                        wd = stage_w[ot]
                        for ht in (2 * half, 2 * half + 1):
                            nc.tensor.matmul(
                                pp[:, :], wd[:, ht, :], ctxn[:, ht, :],
                                start=(ht == 0), stop=(ht == 3))
                        if half == 1:
                            nc.vector.tensor_copy(partialA[:, ot, :], pp[:, :])
                    return f

                # prefetch the first two wd4 chunks from mid-list (DMA only,
                # no PE work) so the first stage matmuls don't stall
                b_units.insert(27, (0, None, lambda: (stage_dma(0), stage_dma(1))))
                # cross out-proj ht 0-3 per o-tile; needs ctxn pairs 0-1
                # (ht 0,1) and 2-3, so gate on min_pair 4 for simplicity
                for ot in range(NT):
                    b_units.append((4, None, mk_stage(ot, 0)))
                    b_units.append((4, None, mk_stage(ot, 1)))

                pre_wd = {}

                def pre_projB():
                    for ot in (0, 1):
                        wd = wdp4.tile([P, 4, P], BF16, tag="wd4",
                                       name=f"wdB{ot}")
                        dma(out=wd[:, :, :], in_=wchunk_view(wdT, ot)[:, 4:NT, :])
                        pre_wd[ot] = wd

                b_units.append((6, None, pre_projB))
                attention(qb, msc, b_units, "B", offset=1)
                proj_ln(slfT, slfT, [], "B", partial=partialA, store=out_d,
                        tail_split=True, pre_wd=pre_wd)

            wvp.release()
            wdp4.release()
            wdp.release()

    nc.compile()
    return nc


_NC = None
import ml_dtypes

BF = ml_dtypes.bfloat16


def make_in_maps(encoder_states, decoder_inputs, src_attention_mask,
                 tgt_attention_mask, Wq, bq, Wk, bk, Wv, bv, Wd, bd, ln_g, ln_b):
    f = np.float32

    def wtile(w):  # [o,i] -> W.T tiled [ot, p, it, c], bf16
        return np.ascontiguousarray(
            np.asarray(w, f).T.reshape(NT, P, NT, P).transpose(2, 1, 0, 3)).astype(BF)

    def atile(x):  # [t,i] -> x.T tiled [p, it, t], bf16
        return np.ascontiguousarray(
            np.asarray(x, f).T.reshape(NT, P, -1).transpose(1, 0, 2)).astype(BF)

    wqT, wkT, wvT, wdT = wtile(Wq), wtile(Wk), wtile(Wv), wtile(Wd)
    col = lambda x: np.ascontiguousarray(
        np.asarray(x, f).reshape(NT, P).T.reshape(P, NT, 1))
    bq_, bk_, bv_, bd_ = col(bq), col(bk), col(bv), col(bd)
    g_, b_ = col(ln_g), col(ln_b)

    decT_b = [atile(decoder_inputs[b]) for b in range(B)]
    encT_b = [atile(encoder_states[b]) for b in range(B)]
    mt_b = [col(tgt_attention_mask[b, 0, 0, :]) for b in range(B)]
    ms_b = [col(src_attention_mask[b, 0, 0, :]) for b in range(B)]

    in_maps = []
    for c in range(8):
        b, half = c // 2, c % 2
        in_maps.append({
            "decT": decT_b[b],
            "decqT": np.ascontiguousarray(decT_b[b][:, :, half * R:(half + 1) * R]),
            "encT": encT_b[b],
            "wqT": wqT, "wkT": wkT, "wvT": wvT, "wdT": wdT,
            "bq": bq_, "bk": bk_, "bv": bv_, "bd": bd_,
            "lng": g_, "lnb": b_,
            "mt": mt_b[b], "ms": ms_b[b],
        })
    return in_maps


def kernel(**inputs):
    global _NC
    if _NC is None:
        _NC = build_kernel()
    nc = _NC
    in_maps = make_in_maps(**inputs)
    res = run_bass_kernel_spmd(nc, in_maps, core_ids=list(range(8)))
    out = np.empty((B, T, H), np.float32)
    for c in range(8):
        b, half = c // 2, c % 2
        buf = res.results[c]["out"]  # [p, ot, t]
        out[b, half * R:(half + 1) * R, :] = (
            buf.transpose(2, 1, 0).reshape(R, H))
    return out
